# revision 29
# baseline (speedup 1.0000x reference)
"""DiffAttn kernel for 8 trn2 NeuronCores (v2).

Problem (per reference):
  X [4, 4096, 1024]; Wq/Wk [1024, 256]; Wv [1024, 128]; biases; lam scalar.
  Q,K = X@Wq+bq, X@Wk+bk ; V = X@Wv+bv
  A_i = Q_i @ K_i^T / sqrt(128)  (i = 1,2 : the two 128-wide halves)
  out = (softmax(A1) - lam * softmax(A2)) @ V          -> [4, 4096, 128]

Sharding: 8 cores = 4 batches x 2 query-halves; each core owns 2048 queries
of one batch and computes K/V for all 4096 keys of that batch redundantly.

v2 dataflow (f32r matmuls, scores kept transposed S^T[key, query]):
  - Projection phase is merged with the first attention pass (comp1/sup0):
    per 512-column X^T chunk, project K1/K2/Q1/Q2/V^T (PSUM rotation,
    DVE bias-evacuation into SBUF), then run 4 attention iterations whose
    key tiles the previous chunk produced. V^T is PE-transposed to V[key,d];
    its bias is folded into the output epilogue ((1-lam)*bv).
  - Attention runs component-sequentially (softmax1 pass, then softmax2
    pass) per 1024-query super-chunk, with "early-exp" issue order:
    exp(k-1) is queued before scores(k) so the PE never head-blocks on the
    single score PSUM bank pair; PV(k-1) trails by one iteration.
  - Softmax denominators: the exp tiles are accumulated into a fp32 pacc
    (alternating DVE/GpSimd to balance engine load), reduced across
    partitions with gpsimd partition_all_reduce, inverted with
    reciprocal_approx_fast, and applied as O1*ib1 - lam*(O2*ib2) + (1-lam)bv
    directly from the PV PSUM accumulators (frees banks for the next pass).
  - Output ships as O^T [128, 2048]; the host transposes (pure layout).
"""

import os
import sys

sys.path.insert(0, "/opt/trn_rl_repo")

import numpy as np

import concourse.bacc as bacc
import concourse.mybir as mybir
from concourse import bass_isa, masks
from concourse.tile import TileContext
from concourse.bass_utils import run_bass_kernel_spmd

F32 = mybir.dt.float32
MM_MODE = os.environ.get("KERNEL_MM_DT", "f32r")
MM_F32R = MM_MODE == "f32r"
MM_DT = mybir.dt.float32r if MM_F32R else F32
AF = mybir.ActivationFunctionType
ALU = mybir.AluOpType

D = 128
EMB = 1024
B, S = 4, 4096
NQ = S // 2          # queries per core
SQC = 512            # projection column chunk
NCC = S // SQC       # 8 projection column chunks
NE = EMB // 128      # 8 emb tiles
SUP = 1024           # attention query super-chunk
NSUP = NQ // SUP     # 2
NSK = S // 128       # 32 key tiles
INV_SQRT_D = 1.0 / np.sqrt(np.float32(D))

TRACE = False
TRACE_DIR = None
LAST_RESULT = None


def _in(ap):
    """Bitcast a DRAM fp32 AP for DMA into an MM_DT tile."""
    return ap.bitcast(MM_DT) if MM_F32R else ap


def _f32(ap):
    return ap.bitcast(F32) if MM_F32R else ap


def _build():
    nc = bacc.Bacc("TRN2", target_bir_lowering=False, debug=False, num_devices=8)

    xt = nc.dram_tensor("xt", [EMB, S], F32, kind="ExternalInput")
    wq = nc.dram_tensor("wq", [EMB, 2 * D], F32, kind="ExternalInput")
    wk = nc.dram_tensor("wk", [EMB, 2 * D], F32, kind="ExternalInput")
    wv = nc.dram_tensor("wv", [EMB, D], F32, kind="ExternalInput")
    bq = nc.dram_tensor("bq", [2 * D, 1], F32, kind="ExternalInput")
    bk = nc.dram_tensor("bk", [2 * D, 1], F32, kind="ExternalInput")
    bv = nc.dram_tensor("bv", [D, 1], F32, kind="ExternalInput")
    lamv = nc.dram_tensor("lamv", [128, 1], F32, kind="ExternalInput")
    out = nc.dram_tensor("o", [D, NQ], F32, kind="ExternalOutput")  # O^T

    from contextlib import ExitStack

    with TileContext(nc) as tc, ExitStack() as ctx:
        # ---------------- SBUF pools ----------------
        cpool = ctx.enter_context(tc.tile_pool(name="const", bufs=1))
        wpool = ctx.enter_context(tc.tile_pool(name="w", bufs=1))
        qkv = ctx.enter_context(tc.tile_pool(name="qkv", bufs=1))
        xpool = ctx.enter_context(tc.tile_pool(name="xt", bufs=2))
        epool = ctx.enter_context(tc.tile_pool(name="e", bufs=5))
        papool = ctx.enter_context(tc.tile_pool(name="pacc", bufs=1))
        fpool = ctx.enter_context(tc.tile_pool(name="fin", bufs=2))
        fpool1 = ctx.enter_context(tc.tile_pool(name="fin1", bufs=1))
        vspool = ctx.enter_context(tc.tile_pool(name="vts", bufs=2))

        # ---------------- constants / biases ----------------
        ident = cpool.tile([128, 128], F32)
        masks.make_identity(nc, ident[:])
        bq1 = cpool.tile([128, 1], F32, tag="bq1")
        bq2 = cpool.tile([128, 1], F32, tag="bq2")
        bk1 = cpool.tile([128, 1], F32, tag="bk1")
        bk2 = cpool.tile([128, 1], F32, tag="bk2")
        bvt = cpool.tile([128, 1], F32, tag="bvt")
        lam_t = cpool.tile([128, 1], F32, tag="lam")
        nc.gpsimd.dma_start(out=bq1[:], in_=bq[0:128, :])
        nc.gpsimd.dma_start(out=bq2[:], in_=bq[128:256, :])
        nc.gpsimd.dma_start(out=bk1[:], in_=bk[0:128, :])
        nc.gpsimd.dma_start(out=bk2[:], in_=bk[128:256, :])
        nc.gpsimd.dma_start(out=bvt[:], in_=bv[0:128, :])
        nc.gpsimd.dma_start(out=lam_t[:], in_=lamv[:, :])

        ones_f = cpool.tile([128, 1], F32, tag="ones_f")
        nc.vector.memset(ones_f[:], 1.0)
        ones_rf = cpool.tile([1, 128], F32, tag="ones_rf")
        nc.vector.memset(ones_rf[:], 1.0)
        ones_col = cpool.tile([128, 1], MM_DT, tag="ones_col")
        nc.vector.tensor_copy(ones_col[:], ones_f[:])
        ones_row = cpool.tile([1, 128], MM_DT, tag="ones_row")
        nc.vector.tensor_copy(ones_row[:], ones_rf[:])

        # epilogue scalars: neg_lam = -lam ; bvl = (1-lam)*bv
        neg_lam = cpool.tile([128, 1], F32, tag="neg_lam")
        oml = cpool.tile([128, 1], F32, tag="oml")
        bvl = cpool.tile([128, 1], F32, tag="bvl")
        nc.vector.tensor_scalar_mul(neg_lam[:], lam_t[:], -1.0)
        nc.vector.tensor_scalar(oml[:], lam_t[:], -1.0, 1.0, ALU.mult, ALU.add)
        nc.vector.tensor_mul(bvl[:], bvt[:], oml[:])

        # ---------------- weights ----------------
        # wk1 as 8 separate slice tiles so the very first matmul only waits
        # for one 64KB weight slice + one 256KB X slice.
        wk1s = [
            wpool.tile([128, 128], MM_DT, tag=f"wk1_{e}", name=f"wk1_{e}")
            for e in range(NE)
        ]
        wq1 = wpool.tile([128, NE, 128], MM_DT, tag="wq1")
        wq2 = wpool.tile([128, NE, 128], MM_DT, tag="wq2")
        wk2 = wpool.tile([128, NE, 128], MM_DT, tag="wk2")
        wvt = wpool.tile([128, NE, 128], MM_DT, tag="wvt")

        def wsrc(w, dsl):
            return _in(w[:, dsl]).rearrange("(t p) d -> p t d", p=128)

        qt1 = qkv.tile([128, NQ], MM_DT, tag="qt1")
        qt2 = qkv.tile([128, NQ], MM_DT, tag="qt2")
        kt1 = qkv.tile([128, S], MM_DT, tag="kt1")
        kt2 = qkv.tile([128, S], MM_DT, tag="kt2")
        vv = qkv.tile([128, S], MM_DT, tag="vv")  # V[key, d]

        # ---------------- attention state ----------------
        st = {}

        def attn_begin(sup, comp, spool):
            st[(sup, comp)] = dict(
                o=opool.tile([128, SUP], F32, tag="o", name=f"o{comp}_{sup}"),
                # independent per-engine accumulators: a single pacc would
                # serialize DVE->Pool->DVE with ~0.3us cross-engine latency
                # per link, pacing the whole pass
                pacc_d=papool.tile(
                    [128, SUP], MM_DT, tag=f"pd{comp}", name=f"pd{comp}_{sup}"
                ),
                pacc_p=papool.tile(
                    [128, SUP], MM_DT, tag=f"pp{comp}", name=f"pp{comp}_{sup}"
                ),
                spool=spool,
                s_prev=None,
                e_prev=None,
            )

        def attn_scores(sup, comp, k):
            ktc, qtc = (kt1, qt1) if comp == 1 else (kt2, qt2)
            s_t = st[(sup, comp)]
            # early-exp: consume the previous score tile before reallocating
            if s_t["s_prev"] is not None:
                e_t = epool.tile([128, SUP], MM_DT, tag="e", name=f"e{comp}_{sup}_{k-1}")
                nc.scalar.activation(
                    e_t[:], s_t["s_prev"][:], AF.Exp, scale=float(INV_SQRT_D)
                )
                s_t["e_prev"] = e_t
            s = s_t["spool"].tile(
                [128, SUP], F32, tag="s", name=f"s{comp}_{sup}_{k}"
            )
            ksl = slice(k * 128, (k + 1) * 128)
            for h in range(2):
                hsl = slice(h * 512, (h + 1) * 512)
                nc.tensor.matmul(
                    s[:, hsl],
                    ktc[:, ksl],
                    qtc[:, sup * SUP + h * 512 : sup * SUP + (h + 1) * 512],
                    start=True,
                    stop=True,
                )
            s_t["s_prev"] = s

        def attn_consume(sup, comp, k):
            """accumulate + PV for iteration k (e_prev must hold exp(k))."""
            s_t = st[(sup, comp)]
            e_t = s_t["e_prev"]
            if k == 0:
                nc.vector.tensor_copy(s_t["pacc_d"][:], _f32(e_t[:]))
            elif k % 5 != 0:
                nc.vector.tensor_add(
                    s_t["pacc_d"][:], _f32(s_t["pacc_d"][:]), _f32(e_t[:])
                )
            elif k == 5:
                nc.gpsimd.tensor_copy(s_t["pacc_p"][:], _f32(e_t[:]))
            else:
                nc.gpsimd.tensor_add(
                    s_t["pacc_p"][:], _f32(s_t["pacc_p"][:]), _f32(e_t[:])
                )
            ksl = slice(k * 128, (k + 1) * 128)
            o_ps = s_t["o"]
            for h in range(2):
                hsl = slice(h * 512, (h + 1) * 512)
                nc.tensor.matmul(
                    o_ps[:, hsl],
                    vv[:, ksl],
                    e_t[:, hsl],
                    start=(k == 0),
                    stop=(k == NSK - 1),
                )

        def attn_drain(sup, comp):
            s_t = st[(sup, comp)]
            e_t = epool.tile([128, SUP], MM_DT, tag="e", name=f"e{comp}_{sup}_31")
            nc.scalar.activation(
                e_t[:], s_t["s_prev"][:], AF.Exp, scale=float(INV_SQRT_D)
            )
            s_t["e_prev"] = e_t
            s_t["s_prev"] = None
            attn_consume(sup, comp, NSK - 1)

        # finalize steps (trickled into the next pass); denominators via
        # fp32 ones-matmuls into spare score-pool PSUM slots (a gpsimd
        # partition_all_reduce needs MODIFY_POOL_CONFIG barriers that stall
        # every engine for ~11us)
        fin = {}

        def fin_evac(sup, comp):
            """Copy the PV accumulator out of PSUM so the o slot frees."""
            s_t = st[(sup, comp)]
            o_s = fpool.tile([128, SUP], F32, tag="oev", name=f"oev{comp}_{sup}")
            nc.vector.tensor_copy(o_s[:], s_t["o"][:])
            s_t["o_s"] = o_s

        def fin_rs(sup, comp, spool):
            s_t = st[(sup, comp)]
            rs = spool.tile([1, SUP], F32, tag="s", name=f"rs{comp}_{sup}")
            for h in range(2):
                hsl = slice(h * 512, (h + 1) * 512)
                nc.tensor.matmul(
                    rs[0:1, hsl], ones_col[:], s_t["pacc_d"][:, hsl],
                    start=True, stop=False,
                )
                nc.tensor.matmul(
                    rs[0:1, hsl], ones_col[:], s_t["pacc_p"][:, hsl],
                    start=False, stop=True,
                )
            fin[(sup, comp, "rs")] = rs

        def fin_recip(sup, comp):
            rs = fin.pop((sup, comp, "rs"))
            r = fpool1.tile([1, SUP], F32, tag="r", name=f"r{comp}_{sup}")
            nc.vector.reciprocal_approx_fast(out=r[0:1, :], in_=rs[0:1, :])
            rr = fpool1.tile([1, SUP], MM_DT, tag="rr", name=f"rr{comp}_{sup}")
            nc.vector.tensor_copy(rr[0:1, :], r[0:1, :])
            fin[(sup, comp, "r")] = rr

        def fin_ib(sup, comp, spool):
            r = fin.pop((sup, comp, "r"))
            ib = spool.tile([128, SUP], F32, tag="s", name=f"ib{comp}_{sup}")
            for h in range(2):
                hsl = slice(h * 512, (h + 1) * 512)
                nc.tensor.matmul(
                    ib[:, hsl], ones_row[:], r[0:1, hsl], start=True, stop=True
                )
            fin[(sup, comp, "ib")] = ib

        def fin_t(sup, comp):
            ib = fin.pop((sup, comp, "ib"))
            s_t = st.pop((sup, comp))
            t = fpool1.tile([128, SUP], F32, tag=f"t{comp}", name=f"t{comp}_{sup}")
            nc.vector.tensor_mul(t[:], s_t["o_s"][:], ib[:])
            if comp == 1:
                # fold the V-bias term (1-lam)*bv here, off the critical tail
                nc.vector.tensor_scalar_add(t[:], t[:], bvl[:, 0:1])
            fin[(sup, comp, "t")] = t

        def fin_out(sup):
            t1 = fin.pop((sup, 1, "t"))
            t2 = fin.pop((sup, 2, "t"))
            oo = fpool1.tile([128, SUP], F32, tag="oo", name=f"oo_{sup}")
            # oo = (t2 * -lam) + t1   (t1 already carries (1-lam)*bv)
            nc.vector.scalar_tensor_tensor(
                out=oo[:],
                in0=t2[:],
                scalar=neg_lam[:, 0:1],
                in1=t1[:],
                op0=ALU.mult,
                op1=ALU.add,
            )
            nc.sync.dma_start(
                out=out[:, sup * SUP : (sup + 1) * SUP], in_=oo[:]
            )

        # ---------------- merged proj + attention ----------------
        opool = ctx.enter_context(tc.tile_pool(name="op", bufs=1, space="PSUM"))

        with ExitStack() as pctx:
            spoolM = pctx.enter_context(tc.tile_pool(name="sM", bufs=1, space="PSUM"))
            ppool = pctx.enter_context(tc.tile_pool(name="pj", bufs=2, space="PSUM"))
            tpool = pctx.enter_context(tc.tile_pool(name="tr", bufs=2, space="PSUM"))
            x0pool = pctx.enter_context(tc.tile_pool(name="x0", bufs=1))

            # startup: wk1 slices + first-chunk slices, finest first
            x0s = [
                x0pool.tile([128, SQC], MM_DT, tag=f"x0_{e}", name=f"x0_{e}")
                for e in range(NE)
            ]
            for e in range(NE):
                r = slice(e * 128, (e + 1) * 128)
                nc.sync.dma_start(out=wk1s[e][:], in_=_in(wk[r, 0:128]))
                nc.sync.dma_start(out=x0s[e][:], in_=_in(xt[r, 0:SQC]))
            nc.gpsimd.dma_start(out=wk2[:], in_=wsrc(wk, slice(128, 256)))
            nc.gpsimd.dma_start(out=wq1[:], in_=wsrc(wq, slice(0, 128)))
            nc.gpsimd.dma_start(out=wvt[:], in_=wsrc(wv, slice(0, 128)))
            nc.gpsimd.dma_start(out=wq2[:], in_=wsrc(wq, slice(128, 256)))

            attn_begin(0, 1, spoolM)

            def xsl(xt_t, e):
                return xt_t[e][:, :] if isinstance(xt_t, list) else xt_t[:, e, :]

            def wslice(w_t, e):
                return w_t[e][:, :] if isinstance(w_t, list) else w_t[:, e, :]

            def proj_group(xt_t, dst, w_t, b_t, tag, cc, csl):
                ps = ppool.tile([128, SQC], F32, tag="pj", name=f"ps_{tag}_{cc}")
                for t in range(NE):
                    nc.tensor.matmul(
                        ps[:],
                        wslice(w_t, t),
                        xsl(xt_t, t),
                        start=(t == 0),
                        stop=(t == NE - 1),
                    )
                nc.scalar.activation(dst[:, csl], ps[:], AF.Identity, bias=b_t[:, 0:1])

            def proj_v(xt_t, cc, csl):
                ps = ppool.tile([128, SQC], F32, tag="pj", name=f"ps_vt_{cc}")
                for t in range(NE):
                    nc.tensor.matmul(
                        ps[:],
                        wslice(wvt, t),
                        xsl(xt_t, t),
                        start=(t == 0),
                        stop=(t == NE - 1),
                    )
                vt_s = vspool.tile([128, SQC], F32, tag="vts", name=f"vts_{cc}")
                nc.scalar.copy(vt_s[:], ps[:])
                for j in range(SQC // 128):
                    tr = tpool.tile([128, 128], F32, tag="vtr", name=f"vtr_{cc}_{j}")
                    nc.tensor.transpose(
                        tr[:], vt_s[:, j * 128 : (j + 1) * 128], ident[:]
                    )
                    col = (cc * (SQC // 128) + j) * 128
                    nc.scalar.copy(vv[:, col : col + 128], tr[:])

            xts = {0: x0s}

            def issue_chunk_dma(cc):
                # prefetch one chunk ahead (xpool bufs=2 bounds the depth)
                if cc >= NCC or cc in xts:
                    return
                t = xpool.tile([128, NE, SQC], MM_DT, tag="xchunk", name=f"xc_{cc}")
                csl_c = slice(cc * SQC, (cc + 1) * SQC)
                nc.sync.dma_start(
                    out=t[:],
                    in_=_in(xt[:, csl_c]).rearrange("(t p) s -> p t s", p=128),
                )
                xts[cc] = t

            issue_chunk_dma(1)
            for cc in range(NCC):
                csl = slice(cc * SQC, (cc + 1) * SQC)
                xt_t = xts.pop(cc)
                issue_chunk_dma(cc + 1)

                units = [
                    lambda t=t: proj_group(xt_t, *t, cc, csl)
                    for t in (
                        [(kt1, wk1s, bk1, "k1"), (kt2, wk2, bk2, "k2")]
                        + (
                            [(qt1, wq1, bq1, "q1"), (qt2, wq2, bq2, "q2")]
                            if cc < NQ // SQC
                            else []
                        )
                    )
                ] + [lambda: proj_v(xt_t, cc, csl)]

                def attn_iter(k):
                    attn_scores(0, 1, k)
                    if k > 0:
                        attn_consume(0, 1, k - 1)

                iters = (
                    [lambda k=k: attn_iter(k) for k in range(4 * (cc - 1), 4 * cc)]
                    if cc >= 1
                    else []
                )
                # interleave proj units with attention iterations; for cc==1
                # the q1/q2 evacuations must precede the first scores that
                # read them, so attention starts only after three units
                lead = 3 if cc == 1 else 1
                seq = units[:lead]
                rest = units[lead:]
                for j in range(max(len(rest), len(iters))):
                    if j < len(iters):
                        seq.append(iters[j])
                    if j < len(rest):
                        seq.append(rest[j])
                for f in seq:
                    f()

            for k in range(4 * (NCC - 1), NSK):
                attn_scores(0, 1, k)
                attn_consume(0, 1, k - 1)
            attn_drain(0, 1)

        # ---------------- remaining passes ----------------
        # triple-buffered score PSUM shared by all passes: scores(k) only
        # waits on exp(k-3), keeping the PE ahead of the ACT queue
        spoolS = ctx.enter_context(tc.tile_pool(name="sS", bufs=3, space="PSUM"))

        def run_pass(sup, comp, prev, last=False):
            # free the previous pass's o slot before reallocating it
            if prev is not None:
                fin_evac(*prev)
            attn_begin(sup, comp, spoolS)
            trickle = (
                {
                    3: lambda: fin_rs(*prev, spoolS),
                    5: lambda: fin_recip(*prev),
                    8: lambda: fin_ib(*prev, spoolS),
                    10: lambda: fin_t(*prev),
                    12: (lambda: fin_out(prev[0])) if prev[1] == 2 else None,
                }
                if prev is not None
                else {}
            )
            for k in range(NSK):
                attn_scores(sup, comp, k)
                if k > 0:
                    attn_consume(sup, comp, k - 1)
                f = trickle.get(k)
                if f is not None:
                    f()
            attn_drain(sup, comp)

        run_pass(0, 2, prev=(0, 1))  # phase A; finalize comp1/sup0 behind it
        run_pass(1, 1, prev=(0, 2))  # phase B; finalize comp2/sup0 + output
        run_pass(1, 2, prev=(1, 1))  # phase C; finalize comp1/sup1
        # tail: finalize comp2/sup1 in independent halves so the serial
        # rs->recip->ib->mul->sub chain pipelines across PE/DVE
        s_t2 = st.pop((1, 2))
        o_s2 = fpool.tile([128, SUP], F32, tag="oev", name="oev2_1t")
        t1f = fin.pop((1, 1, "t"))
        rs_t = spoolS.tile([1, SUP], F32, tag="s", name="rs_tail")
        ib_t = spoolS.tile([128, SUP], F32, tag="s", name="ib_tail")
        for h in range(2):
            hsl = slice(h * 512, (h + 1) * 512)
            nc.vector.tensor_copy(o_s2[:, hsl], s_t2["o"][:, hsl])
            nc.tensor.matmul(
                rs_t[0:1, hsl], ones_col[:], s_t2["pacc_d"][:, hsl],
                start=True, stop=False,
            )
            nc.tensor.matmul(
                rs_t[0:1, hsl], ones_col[:], s_t2["pacc_p"][:, hsl],
                start=False, stop=True,
            )
            r_h = fpool1.tile([1, 512], F32, tag="r", name=f"rt_{h}")
            nc.vector.reciprocal_approx_fast(out=r_h[0:1, :], in_=rs_t[0:1, hsl])
            rr_h = fpool1.tile([1, 512], MM_DT, tag="rr", name=f"rrt_{h}")
            nc.vector.tensor_copy(rr_h[0:1, :], r_h[0:1, :])
            nc.tensor.matmul(
                ib_t[:, hsl], ones_row[:], rr_h[0:1, :], start=True, stop=True
            )
            t2_h = fpool1.tile([128, 512], F32, tag="t2", name=f"t2t_{h}")
            nc.vector.tensor_mul(t2_h[:], o_s2[:, hsl], ib_t[:, hsl])
            oo_h = fpool1.tile([128, 512], F32, tag="oo", name=f"oot_{h}")
            nc.vector.scalar_tensor_tensor(
                out=oo_h[:],
                in0=t2_h[:],
                scalar=neg_lam[:, 0:1],
                in1=t1f[:, hsl],
                op0=ALU.mult,
                op1=ALU.add,
            )
            nc.sync.dma_start(
                out=out[:, SUP + h * 512 : SUP + (h + 1) * 512], in_=oo_h[:]
            )

    nc.compile()
    return nc


_NC = None


def _get_nc():
    global _NC
    if _NC is None:
        _NC = _build()
    return _NC


def kernel(X, lam, Wq, bq, Wk, bk, Wv, bv):
    X = np.asarray(X, dtype=np.float32)
    lam_f = float(np.asarray(lam))
    Wq = np.ascontiguousarray(np.asarray(Wq, np.float32))
    Wk = np.ascontiguousarray(np.asarray(Wk, np.float32))
    Wv = np.ascontiguousarray(np.asarray(Wv, np.float32))
    bq_c = np.asarray(bq, np.float32).reshape(2 * D, 1).copy()
    bk_c = np.asarray(bk, np.float32).reshape(2 * D, 1).copy()
    bv_c = np.asarray(bv, np.float32).reshape(D, 1).copy()
    lam_v = np.full((128, 1), lam_f, np.float32)

    nc = _get_nc()

    in_maps = []
    for core in range(8):
        b, h = divmod(core, 2)
        xb = X[b]
        if h == 0:
            xr = xb
        else:
            xr = np.concatenate([xb[NQ:], xb[:NQ]], axis=0)
        xt_a = np.ascontiguousarray(xr.T)
        in_maps.append(
            {
                "xt": xt_a,
                "wq": Wq,
                "wk": Wk,
                "wv": Wv,
                "bq": bq_c,
                "bk": bk_c,
                "bv": bv_c,
                "lamv": lam_v,
            }
        )

    global LAST_RESULT
    kwargs = {}
    if TRACE:
        import tempfile

        tdir = tempfile.mkdtemp(dir=TRACE_DIR) if TRACE_DIR else None
        kwargs = dict(trace=True, tmpdir=tdir)
    res = run_bass_kernel_spmd(nc, in_maps, list(range(8)), **kwargs)
    LAST_RESULT = res

    o = np.empty((B, S, D), np.float32)
    for core in range(8):
        b, h = divmod(core, 2)
        o[b, h * NQ : (h + 1) * NQ, :] = res.results[core]["o"].T
    return o


# revision 30
# speedup vs baseline: 1.0043x; 1.0043x over previous
"""DiffAttn kernel for 8 trn2 NeuronCores (v2).

Problem (per reference):
  X [4, 4096, 1024]; Wq/Wk [1024, 256]; Wv [1024, 128]; biases; lam scalar.
  Q,K = X@Wq+bq, X@Wk+bk ; V = X@Wv+bv
  A_i = Q_i @ K_i^T / sqrt(128)  (i = 1,2 : the two 128-wide halves)
  out = (softmax(A1) - lam * softmax(A2)) @ V          -> [4, 4096, 128]

Sharding: 8 cores = 4 batches x 2 query-halves; each core owns 2048 queries
of one batch and computes K/V for all 4096 keys of that batch redundantly.

v2 dataflow (f32r matmuls, scores kept transposed S^T[key, query]):
  - Projection phase is merged with the first attention pass (comp1/sup0):
    per 512-column X^T chunk, project K1/K2/Q1/Q2/V^T (PSUM rotation,
    DVE bias-evacuation into SBUF), then run 4 attention iterations whose
    key tiles the previous chunk produced. V^T is PE-transposed to V[key,d];
    its bias is folded into the output epilogue ((1-lam)*bv).
  - Attention runs component-sequentially (softmax1 pass, then softmax2
    pass) per 1024-query super-chunk, with "early-exp" issue order:
    exp(k-1) is queued before scores(k) so the PE never head-blocks on the
    single score PSUM bank pair; PV(k-1) trails by one iteration.
  - Softmax denominators: the exp tiles are accumulated into a fp32 pacc
    (alternating DVE/GpSimd to balance engine load), reduced across
    partitions with gpsimd partition_all_reduce, inverted with
    reciprocal_approx_fast, and applied as O1*ib1 - lam*(O2*ib2) + (1-lam)bv
    directly from the PV PSUM accumulators (frees banks for the next pass).
  - Output ships as O^T [128, 2048]; the host transposes (pure layout).
"""

import os
import sys

sys.path.insert(0, "/opt/trn_rl_repo")

import numpy as np

import concourse.bacc as bacc
import concourse.mybir as mybir
from concourse import bass_isa, masks
from concourse.tile import TileContext
from concourse.bass_utils import run_bass_kernel_spmd

F32 = mybir.dt.float32
MM_MODE = os.environ.get("KERNEL_MM_DT", "f32r")
MM_F32R = MM_MODE == "f32r"
MM_DT = mybir.dt.float32r if MM_F32R else F32
AF = mybir.ActivationFunctionType
ALU = mybir.AluOpType

D = 128
EMB = 1024
B, S = 4, 4096
NQ = S // 2          # queries per core
SQC = 512            # projection column chunk
NCC = S // SQC       # 8 projection column chunks
NE = EMB // 128      # 8 emb tiles
SUP = 1024           # attention query super-chunk
NSUP = NQ // SUP     # 2
NSK = S // 128       # 32 key tiles
INV_SQRT_D = 1.0 / np.sqrt(np.float32(D))

TRACE = False
TRACE_DIR = None
LAST_RESULT = None


def _in(ap):
    """Bitcast a DRAM fp32 AP for DMA into an MM_DT tile."""
    return ap.bitcast(MM_DT) if MM_F32R else ap


def _f32(ap):
    return ap.bitcast(F32) if MM_F32R else ap


def _build():
    nc = bacc.Bacc("TRN2", target_bir_lowering=False, debug=False, num_devices=8)

    xt = nc.dram_tensor("xt", [EMB, S], F32, kind="ExternalInput")
    wq = nc.dram_tensor("wq", [EMB, 2 * D], F32, kind="ExternalInput")
    wk = nc.dram_tensor("wk", [EMB, 2 * D], F32, kind="ExternalInput")
    wv = nc.dram_tensor("wv", [EMB, D], F32, kind="ExternalInput")
    bq = nc.dram_tensor("bq", [2 * D, 1], F32, kind="ExternalInput")
    bk = nc.dram_tensor("bk", [2 * D, 1], F32, kind="ExternalInput")
    bv = nc.dram_tensor("bv", [D, 1], F32, kind="ExternalInput")
    lamv = nc.dram_tensor("lamv", [128, 1], F32, kind="ExternalInput")
    out = nc.dram_tensor("o", [D, NQ], F32, kind="ExternalOutput")  # O^T

    from contextlib import ExitStack

    with TileContext(nc) as tc, ExitStack() as ctx:
        # ---------------- SBUF pools ----------------
        cpool = ctx.enter_context(tc.tile_pool(name="const", bufs=1))
        wpool = ctx.enter_context(tc.tile_pool(name="w", bufs=1))
        qkv = ctx.enter_context(tc.tile_pool(name="qkv", bufs=1))
        xpool = ctx.enter_context(tc.tile_pool(name="xt", bufs=2))
        epool = ctx.enter_context(tc.tile_pool(name="e", bufs=5))
        papool = ctx.enter_context(tc.tile_pool(name="pacc", bufs=1))
        fpool = ctx.enter_context(tc.tile_pool(name="fin", bufs=2))
        fpool1 = ctx.enter_context(tc.tile_pool(name="fin1", bufs=1))
        vspool = ctx.enter_context(tc.tile_pool(name="vts", bufs=2))

        # ---------------- constants / biases ----------------
        ident = cpool.tile([128, 128], F32)
        masks.make_identity(nc, ident[:])
        bq1 = cpool.tile([128, 1], F32, tag="bq1")
        bq2 = cpool.tile([128, 1], F32, tag="bq2")
        bk1 = cpool.tile([128, 1], F32, tag="bk1")
        bk2 = cpool.tile([128, 1], F32, tag="bk2")
        bvt = cpool.tile([128, 1], F32, tag="bvt")
        lam_t = cpool.tile([128, 1], F32, tag="lam")
        nc.gpsimd.dma_start(out=bq1[:], in_=bq[0:128, :])
        nc.gpsimd.dma_start(out=bq2[:], in_=bq[128:256, :])
        nc.gpsimd.dma_start(out=bk1[:], in_=bk[0:128, :])
        nc.gpsimd.dma_start(out=bk2[:], in_=bk[128:256, :])
        nc.gpsimd.dma_start(out=bvt[:], in_=bv[0:128, :])
        nc.gpsimd.dma_start(out=lam_t[:], in_=lamv[:, :])

        ones_f = cpool.tile([128, 1], F32, tag="ones_f")
        nc.vector.memset(ones_f[:], 1.0)
        ones_rf = cpool.tile([1, 128], F32, tag="ones_rf")
        nc.vector.memset(ones_rf[:], 1.0)
        ones_col = cpool.tile([128, 1], MM_DT, tag="ones_col")
        nc.vector.tensor_copy(ones_col[:], ones_f[:])
        ones_row = cpool.tile([1, 128], MM_DT, tag="ones_row")
        nc.vector.tensor_copy(ones_row[:], ones_rf[:])

        # epilogue scalars: neg_lam = -lam ; bvl = (1-lam)*bv
        neg_lam = cpool.tile([128, 1], F32, tag="neg_lam")
        oml = cpool.tile([128, 1], F32, tag="oml")
        bvl = cpool.tile([128, 1], F32, tag="bvl")
        nc.vector.tensor_scalar_mul(neg_lam[:], lam_t[:], -1.0)
        nc.vector.tensor_scalar(oml[:], lam_t[:], -1.0, 1.0, ALU.mult, ALU.add)
        nc.vector.tensor_mul(bvl[:], bvt[:], oml[:])

        # ---------------- weights ----------------
        # wk1 as 8 separate slice tiles so the very first matmul only waits
        # for one 64KB weight slice + one 256KB X slice.
        wk1s = [
            wpool.tile([128, 128], MM_DT, tag=f"wk1_{e}", name=f"wk1_{e}")
            for e in range(NE)
        ]
        wq1 = wpool.tile([128, NE, 128], MM_DT, tag="wq1")
        wq2 = wpool.tile([128, NE, 128], MM_DT, tag="wq2")
        wk2 = wpool.tile([128, NE, 128], MM_DT, tag="wk2")
        wvt = wpool.tile([128, NE, 128], MM_DT, tag="wvt")

        def wsrc(w, dsl):
            return _in(w[:, dsl]).rearrange("(t p) d -> p t d", p=128)

        qt1 = qkv.tile([128, NQ], MM_DT, tag="qt1")
        qt2 = qkv.tile([128, NQ], MM_DT, tag="qt2")
        kt1 = qkv.tile([128, S], MM_DT, tag="kt1")
        kt2 = qkv.tile([128, S], MM_DT, tag="kt2")
        vv = qkv.tile([128, S], MM_DT, tag="vv")  # V[key, d]

        # ---------------- attention state ----------------
        st = {}

        def attn_begin(sup, comp, spool):
            st[(sup, comp)] = dict(
                o=opool.tile([128, SUP], F32, tag="o", name=f"o{comp}_{sup}"),
                # independent per-engine accumulators: a single pacc would
                # serialize DVE->Pool->DVE with ~0.3us cross-engine latency
                # per link, pacing the whole pass
                pacc_d=papool.tile(
                    [128, SUP], MM_DT, tag=f"pd{comp}", name=f"pd{comp}_{sup}"
                ),
                pacc_p=papool.tile(
                    [128, SUP], MM_DT, tag=f"pp{comp}", name=f"pp{comp}_{sup}"
                ),
                spool=spool,
                s_prev=None,
                e_prev=None,
            )

        def attn_scores(sup, comp, k):
            ktc, qtc = (kt1, qt1) if comp == 1 else (kt2, qt2)
            s_t = st[(sup, comp)]
            # early-exp: consume the previous score tile before reallocating
            if s_t["s_prev"] is not None:
                e_t = epool.tile([128, SUP], MM_DT, tag="e", name=f"e{comp}_{sup}_{k-1}")
                nc.scalar.activation(
                    e_t[:], s_t["s_prev"][:], AF.Exp, scale=float(INV_SQRT_D)
                )
                s_t["e_prev"] = e_t
            s = s_t["spool"].tile(
                [128, SUP], F32, tag="s", name=f"s{comp}_{sup}_{k}"
            )
            ksl = slice(k * 128, (k + 1) * 128)
            for h in range(2):
                hsl = slice(h * 512, (h + 1) * 512)
                nc.tensor.matmul(
                    s[:, hsl],
                    ktc[:, ksl],
                    qtc[:, sup * SUP + h * 512 : sup * SUP + (h + 1) * 512],
                    start=True,
                    stop=True,
                )
            s_t["s_prev"] = s

        def attn_consume(sup, comp, k):
            """accumulate + PV for iteration k (e_prev must hold exp(k))."""
            s_t = st[(sup, comp)]
            e_t = s_t["e_prev"]
            if k == 0:
                nc.vector.tensor_copy(s_t["pacc_d"][:], _f32(e_t[:]))
            elif k % 8 != 0:
                nc.vector.tensor_add(
                    s_t["pacc_d"][:], _f32(s_t["pacc_d"][:]), _f32(e_t[:])
                )
            elif k == 8:
                nc.gpsimd.tensor_copy(s_t["pacc_p"][:], _f32(e_t[:]))
            else:
                nc.gpsimd.tensor_add(
                    s_t["pacc_p"][:], _f32(s_t["pacc_p"][:]), _f32(e_t[:])
                )
            ksl = slice(k * 128, (k + 1) * 128)
            o_ps = s_t["o"]
            for h in range(2):
                hsl = slice(h * 512, (h + 1) * 512)
                nc.tensor.matmul(
                    o_ps[:, hsl],
                    vv[:, ksl],
                    e_t[:, hsl],
                    start=(k == 0),
                    stop=(k == NSK - 1),
                )

        def attn_drain(sup, comp):
            s_t = st[(sup, comp)]
            e_t = epool.tile([128, SUP], MM_DT, tag="e", name=f"e{comp}_{sup}_31")
            nc.scalar.activation(
                e_t[:], s_t["s_prev"][:], AF.Exp, scale=float(INV_SQRT_D)
            )
            s_t["e_prev"] = e_t
            s_t["s_prev"] = None
            attn_consume(sup, comp, NSK - 1)

        # finalize steps (trickled into the next pass); denominators via
        # fp32 ones-matmuls into spare score-pool PSUM slots (a gpsimd
        # partition_all_reduce needs MODIFY_POOL_CONFIG barriers that stall
        # every engine for ~11us)
        fin = {}

        def fin_evac(sup, comp):
            """Copy the PV accumulator out of PSUM so the o slot frees."""
            s_t = st[(sup, comp)]
            o_s = fpool.tile([128, SUP], F32, tag="oev", name=f"oev{comp}_{sup}")
            nc.vector.tensor_copy(o_s[:], s_t["o"][:])
            s_t["o_s"] = o_s

        def fin_rs(sup, comp, spool):
            s_t = st[(sup, comp)]
            rs = spool.tile([1, SUP], F32, tag="s", name=f"rs{comp}_{sup}")
            for h in range(2):
                hsl = slice(h * 512, (h + 1) * 512)
                nc.tensor.matmul(
                    rs[0:1, hsl], ones_col[:], s_t["pacc_d"][:, hsl],
                    start=True, stop=False,
                )
                nc.tensor.matmul(
                    rs[0:1, hsl], ones_col[:], s_t["pacc_p"][:, hsl],
                    start=False, stop=True,
                )
            fin[(sup, comp, "rs")] = rs

        def fin_recip(sup, comp):
            rs = fin.pop((sup, comp, "rs"))
            r = fpool1.tile([1, SUP], F32, tag="r", name=f"r{comp}_{sup}")
            nc.vector.reciprocal_approx_fast(out=r[0:1, :], in_=rs[0:1, :])
            rr = fpool1.tile([1, SUP], MM_DT, tag="rr", name=f"rr{comp}_{sup}")
            nc.vector.tensor_copy(rr[0:1, :], r[0:1, :])
            fin[(sup, comp, "r")] = rr

        def fin_ib(sup, comp, spool):
            r = fin.pop((sup, comp, "r"))
            ib = spool.tile([128, SUP], F32, tag="s", name=f"ib{comp}_{sup}")
            for h in range(2):
                hsl = slice(h * 512, (h + 1) * 512)
                nc.tensor.matmul(
                    ib[:, hsl], ones_row[:], r[0:1, hsl], start=True, stop=True
                )
            fin[(sup, comp, "ib")] = ib

        def fin_t(sup, comp):
            ib = fin.pop((sup, comp, "ib"))
            s_t = st.pop((sup, comp))
            t = fpool1.tile([128, SUP], F32, tag=f"t{comp}", name=f"t{comp}_{sup}")
            nc.vector.tensor_mul(t[:], s_t["o_s"][:], ib[:])
            if comp == 1:
                # fold the V-bias term (1-lam)*bv here, off the critical tail
                nc.vector.tensor_scalar_add(t[:], t[:], bvl[:, 0:1])
            fin[(sup, comp, "t")] = t

        def fin_out(sup):
            t1 = fin.pop((sup, 1, "t"))
            t2 = fin.pop((sup, 2, "t"))
            oo = fpool1.tile([128, SUP], F32, tag="oo", name=f"oo_{sup}")
            # oo = (t2 * -lam) + t1   (t1 already carries (1-lam)*bv)
            nc.vector.scalar_tensor_tensor(
                out=oo[:],
                in0=t2[:],
                scalar=neg_lam[:, 0:1],
                in1=t1[:],
                op0=ALU.mult,
                op1=ALU.add,
            )
            nc.sync.dma_start(
                out=out[:, sup * SUP : (sup + 1) * SUP], in_=oo[:]
            )

        # ---------------- merged proj + attention ----------------
        opool = ctx.enter_context(tc.tile_pool(name="op", bufs=1, space="PSUM"))

        with ExitStack() as pctx:
            spoolM = pctx.enter_context(tc.tile_pool(name="sM", bufs=1, space="PSUM"))
            ppool = pctx.enter_context(tc.tile_pool(name="pj", bufs=2, space="PSUM"))
            tpool = pctx.enter_context(tc.tile_pool(name="tr", bufs=2, space="PSUM"))
            x0pool = pctx.enter_context(tc.tile_pool(name="x0", bufs=1))

            # startup: wk1 slices + first-chunk slices, finest first
            x0s = [
                x0pool.tile([128, SQC], MM_DT, tag=f"x0_{e}", name=f"x0_{e}")
                for e in range(NE)
            ]
            for e in range(NE):
                r = slice(e * 128, (e + 1) * 128)
                nc.sync.dma_start(out=wk1s[e][:], in_=_in(wk[r, 0:128]))
                nc.sync.dma_start(out=x0s[e][:], in_=_in(xt[r, 0:SQC]))
            nc.gpsimd.dma_start(out=wk2[:], in_=wsrc(wk, slice(128, 256)))
            nc.gpsimd.dma_start(out=wq1[:], in_=wsrc(wq, slice(0, 128)))
            nc.gpsimd.dma_start(out=wvt[:], in_=wsrc(wv, slice(0, 128)))
            nc.gpsimd.dma_start(out=wq2[:], in_=wsrc(wq, slice(128, 256)))

            attn_begin(0, 1, spoolM)

            def xsl(xt_t, e):
                return xt_t[e][:, :] if isinstance(xt_t, list) else xt_t[:, e, :]

            def wslice(w_t, e):
                return w_t[e][:, :] if isinstance(w_t, list) else w_t[:, e, :]

            def proj_group(xt_t, dst, w_t, b_t, tag, cc, csl):
                ps = ppool.tile([128, SQC], F32, tag="pj", name=f"ps_{tag}_{cc}")
                for t in range(NE):
                    nc.tensor.matmul(
                        ps[:],
                        wslice(w_t, t),
                        xsl(xt_t, t),
                        start=(t == 0),
                        stop=(t == NE - 1),
                    )
                nc.scalar.activation(dst[:, csl], ps[:], AF.Identity, bias=b_t[:, 0:1])

            def proj_v(xt_t, cc, csl):
                ps = ppool.tile([128, SQC], F32, tag="pj", name=f"ps_vt_{cc}")
                for t in range(NE):
                    nc.tensor.matmul(
                        ps[:],
                        wslice(wvt, t),
                        xsl(xt_t, t),
                        start=(t == 0),
                        stop=(t == NE - 1),
                    )
                vt_s = vspool.tile([128, SQC], F32, tag="vts", name=f"vts_{cc}")
                nc.scalar.copy(vt_s[:], ps[:])
                for j in range(SQC // 128):
                    tr = tpool.tile([128, 128], F32, tag="vtr", name=f"vtr_{cc}_{j}")
                    nc.tensor.transpose(
                        tr[:], vt_s[:, j * 128 : (j + 1) * 128], ident[:]
                    )
                    col = (cc * (SQC // 128) + j) * 128
                    nc.scalar.copy(vv[:, col : col + 128], tr[:])

            xts = {0: x0s}

            def issue_chunk_dma(cc):
                # prefetch one chunk ahead (xpool bufs=2 bounds the depth)
                if cc >= NCC or cc in xts:
                    return
                t = xpool.tile([128, NE, SQC], MM_DT, tag="xchunk", name=f"xc_{cc}")
                csl_c = slice(cc * SQC, (cc + 1) * SQC)
                nc.sync.dma_start(
                    out=t[:],
                    in_=_in(xt[:, csl_c]).rearrange("(t p) s -> p t s", p=128),
                )
                xts[cc] = t

            issue_chunk_dma(1)
            for cc in range(NCC):
                csl = slice(cc * SQC, (cc + 1) * SQC)
                xt_t = xts.pop(cc)
                issue_chunk_dma(cc + 1)

                units = [
                    lambda t=t: proj_group(xt_t, *t, cc, csl)
                    for t in (
                        [(kt1, wk1s, bk1, "k1"), (kt2, wk2, bk2, "k2")]
                        + (
                            [(qt1, wq1, bq1, "q1"), (qt2, wq2, bq2, "q2")]
                            if cc < NQ // SQC
                            else []
                        )
                    )
                ] + [lambda: proj_v(xt_t, cc, csl)]

                def attn_iter(k):
                    attn_scores(0, 1, k)
                    if k > 0:
                        attn_consume(0, 1, k - 1)

                iters = (
                    [lambda k=k: attn_iter(k) for k in range(4 * (cc - 1), 4 * cc)]
                    if cc >= 1
                    else []
                )
                # interleave proj units with attention iterations; for cc==1
                # the q1/q2 evacuations must precede the first scores that
                # read them, so attention starts only after three units
                lead = 3 if cc == 1 else 1
                seq = units[:lead]
                rest = units[lead:]
                for j in range(max(len(rest), len(iters))):
                    if j < len(iters):
                        seq.append(iters[j])
                    if j < len(rest):
                        seq.append(rest[j])
                for f in seq:
                    f()

            for k in range(4 * (NCC - 1), NSK):
                attn_scores(0, 1, k)
                attn_consume(0, 1, k - 1)
            attn_drain(0, 1)

        # ---------------- remaining passes ----------------
        # triple-buffered score PSUM shared by all passes: scores(k) only
        # waits on exp(k-3), keeping the PE ahead of the ACT queue
        spoolS = ctx.enter_context(tc.tile_pool(name="sS", bufs=3, space="PSUM"))

        def run_pass(sup, comp, prev, last=False):
            # free the previous pass's o slot before reallocating it
            if prev is not None:
                fin_evac(*prev)
            attn_begin(sup, comp, spoolS)
            trickle = (
                {
                    3: lambda: fin_rs(*prev, spoolS),
                    5: lambda: fin_recip(*prev),
                    8: lambda: fin_ib(*prev, spoolS),
                    10: lambda: fin_t(*prev),
                    12: (lambda: fin_out(prev[0])) if prev[1] == 2 else None,
                }
                if prev is not None
                else {}
            )
            for k in range(NSK):
                attn_scores(sup, comp, k)
                if k > 0:
                    attn_consume(sup, comp, k - 1)
                f = trickle.get(k)
                if f is not None:
                    f()
            attn_drain(sup, comp)

        run_pass(0, 2, prev=(0, 1))  # phase A; finalize comp1/sup0 behind it
        run_pass(1, 1, prev=(0, 2))  # phase B; finalize comp2/sup0 + output
        run_pass(1, 2, prev=(1, 1))  # phase C; finalize comp1/sup1
        # tail: finalize comp2/sup1 in independent halves so the serial
        # rs->recip->ib->mul->sub chain pipelines across PE/DVE
        s_t2 = st.pop((1, 2))
        o_s2 = fpool.tile([128, SUP], F32, tag="oev", name="oev2_1t")
        t1f = fin.pop((1, 1, "t"))
        rs_t = spoolS.tile([1, SUP], F32, tag="s", name="rs_tail")
        ib_t = spoolS.tile([128, SUP], F32, tag="s", name="ib_tail")
        for h in range(2):
            hsl = slice(h * 512, (h + 1) * 512)
            nc.vector.tensor_copy(o_s2[:, hsl], s_t2["o"][:, hsl])
            nc.tensor.matmul(
                rs_t[0:1, hsl], ones_col[:], s_t2["pacc_d"][:, hsl],
                start=True, stop=False,
            )
            nc.tensor.matmul(
                rs_t[0:1, hsl], ones_col[:], s_t2["pacc_p"][:, hsl],
                start=False, stop=True,
            )
            r_h = fpool1.tile([1, 512], F32, tag="r", name=f"rt_{h}")
            nc.vector.reciprocal_approx_fast(out=r_h[0:1, :], in_=rs_t[0:1, hsl])
            rr_h = fpool1.tile([1, 512], MM_DT, tag="rr", name=f"rrt_{h}")
            nc.vector.tensor_copy(rr_h[0:1, :], r_h[0:1, :])
            nc.tensor.matmul(
                ib_t[:, hsl], ones_row[:], rr_h[0:1, :], start=True, stop=True
            )
            t2_h = fpool1.tile([128, 512], F32, tag="t2", name=f"t2t_{h}")
            nc.vector.tensor_mul(t2_h[:], o_s2[:, hsl], ib_t[:, hsl])
            oo_h = fpool1.tile([128, 512], F32, tag="oo", name=f"oot_{h}")
            nc.vector.scalar_tensor_tensor(
                out=oo_h[:],
                in0=t2_h[:],
                scalar=neg_lam[:, 0:1],
                in1=t1f[:, hsl],
                op0=ALU.mult,
                op1=ALU.add,
            )
            nc.sync.dma_start(
                out=out[:, SUP + h * 512 : SUP + (h + 1) * 512], in_=oo_h[:]
            )

    nc.compile()
    return nc


_NC = None


def _get_nc():
    global _NC
    if _NC is None:
        _NC = _build()
    return _NC


def kernel(X, lam, Wq, bq, Wk, bk, Wv, bv):
    X = np.asarray(X, dtype=np.float32)
    lam_f = float(np.asarray(lam))
    Wq = np.ascontiguousarray(np.asarray(Wq, np.float32))
    Wk = np.ascontiguousarray(np.asarray(Wk, np.float32))
    Wv = np.ascontiguousarray(np.asarray(Wv, np.float32))
    bq_c = np.asarray(bq, np.float32).reshape(2 * D, 1).copy()
    bk_c = np.asarray(bk, np.float32).reshape(2 * D, 1).copy()
    bv_c = np.asarray(bv, np.float32).reshape(D, 1).copy()
    lam_v = np.full((128, 1), lam_f, np.float32)

    nc = _get_nc()

    in_maps = []
    for core in range(8):
        b, h = divmod(core, 2)
        xb = X[b]
        if h == 0:
            xr = xb
        else:
            xr = np.concatenate([xb[NQ:], xb[:NQ]], axis=0)
        xt_a = np.ascontiguousarray(xr.T)
        in_maps.append(
            {
                "xt": xt_a,
                "wq": Wq,
                "wk": Wk,
                "wv": Wv,
                "bq": bq_c,
                "bk": bk_c,
                "bv": bv_c,
                "lamv": lam_v,
            }
        )

    global LAST_RESULT
    kwargs = {}
    if TRACE:
        import tempfile

        tdir = tempfile.mkdtemp(dir=TRACE_DIR) if TRACE_DIR else None
        kwargs = dict(trace=True, tmpdir=tdir)
    res = run_bass_kernel_spmd(nc, in_maps, list(range(8)), **kwargs)
    LAST_RESULT = res

    o = np.empty((B, S, D), np.float32)
    for core in range(8):
        b, h = divmod(core, 2)
        o[b, h * NQ : (h + 1) * NQ, :] = res.results[core]["o"].T
    return o


# revision 31
# speedup vs baseline: 1.0046x; 1.0003x over previous
"""DiffAttn kernel for 8 trn2 NeuronCores (v2).

Problem (per reference):
  X [4, 4096, 1024]; Wq/Wk [1024, 256]; Wv [1024, 128]; biases; lam scalar.
  Q,K = X@Wq+bq, X@Wk+bk ; V = X@Wv+bv
  A_i = Q_i @ K_i^T / sqrt(128)  (i = 1,2 : the two 128-wide halves)
  out = (softmax(A1) - lam * softmax(A2)) @ V          -> [4, 4096, 128]

Sharding: 8 cores = 4 batches x 2 query-halves; each core owns 2048 queries
of one batch and computes K/V for all 4096 keys of that batch redundantly.

v2 dataflow (f32r matmuls, scores kept transposed S^T[key, query]):
  - Projection phase is merged with the first attention pass (comp1/sup0):
    per 512-column X^T chunk, project K1/K2/Q1/Q2/V^T (PSUM rotation,
    DVE bias-evacuation into SBUF), then run 4 attention iterations whose
    key tiles the previous chunk produced. V^T is PE-transposed to V[key,d];
    its bias is folded into the output epilogue ((1-lam)*bv).
  - Attention runs component-sequentially (softmax1 pass, then softmax2
    pass) per 1024-query super-chunk, with "early-exp" issue order:
    exp(k-1) is queued before scores(k) so the PE never head-blocks on the
    single score PSUM bank pair; PV(k-1) trails by one iteration.
  - Softmax denominators: the exp tiles are accumulated into a fp32 pacc
    (alternating DVE/GpSimd to balance engine load), reduced across
    partitions with gpsimd partition_all_reduce, inverted with
    reciprocal_approx_fast, and applied as O1*ib1 - lam*(O2*ib2) + (1-lam)bv
    directly from the PV PSUM accumulators (frees banks for the next pass).
  - Output ships as O^T [128, 2048]; the host transposes (pure layout).
"""

import os
import sys

sys.path.insert(0, "/opt/trn_rl_repo")

import numpy as np

import concourse.bacc as bacc
import concourse.mybir as mybir
from concourse import bass_isa, masks
from concourse.tile import TileContext
from concourse.bass_utils import run_bass_kernel_spmd

F32 = mybir.dt.float32
MM_MODE = os.environ.get("KERNEL_MM_DT", "f32r")
MM_F32R = MM_MODE == "f32r"
MM_DT = mybir.dt.float32r if MM_F32R else F32
AF = mybir.ActivationFunctionType
ALU = mybir.AluOpType

D = 128
EMB = 1024
B, S = 4, 4096
NQ = S // 2          # queries per core
SQC = 512            # projection column chunk
NCC = S // SQC       # 8 projection column chunks
NE = EMB // 128      # 8 emb tiles
SUP = 1024           # attention query super-chunk
NSUP = NQ // SUP     # 2
NSK = S // 128       # 32 key tiles
INV_SQRT_D = 1.0 / np.sqrt(np.float32(D))

TRACE = False
TRACE_DIR = None
LAST_RESULT = None


def _in(ap):
    """Bitcast a DRAM fp32 AP for DMA into an MM_DT tile."""
    return ap.bitcast(MM_DT) if MM_F32R else ap


def _f32(ap):
    return ap.bitcast(F32) if MM_F32R else ap


def _build():
    nc = bacc.Bacc("TRN2", target_bir_lowering=False, debug=False, num_devices=8)

    xt = nc.dram_tensor("xt", [EMB, S], F32, kind="ExternalInput")
    wq = nc.dram_tensor("wq", [EMB, 2 * D], F32, kind="ExternalInput")
    wk = nc.dram_tensor("wk", [EMB, 2 * D], F32, kind="ExternalInput")
    wv = nc.dram_tensor("wv", [EMB, D], F32, kind="ExternalInput")
    bq = nc.dram_tensor("bq", [2 * D, 1], F32, kind="ExternalInput")
    bk = nc.dram_tensor("bk", [2 * D, 1], F32, kind="ExternalInput")
    bv = nc.dram_tensor("bv", [D, 1], F32, kind="ExternalInput")
    lamv = nc.dram_tensor("lamv", [128, 1], F32, kind="ExternalInput")
    out = nc.dram_tensor("o", [D, NQ], F32, kind="ExternalOutput")  # O^T

    from contextlib import ExitStack

    with TileContext(nc) as tc, ExitStack() as ctx:
        # ---------------- SBUF pools ----------------
        cpool = ctx.enter_context(tc.tile_pool(name="const", bufs=1))
        wpool = ctx.enter_context(tc.tile_pool(name="w", bufs=1))
        qkv = ctx.enter_context(tc.tile_pool(name="qkv", bufs=1))
        xpool = ctx.enter_context(tc.tile_pool(name="xt", bufs=2))
        epool = ctx.enter_context(tc.tile_pool(name="e", bufs=5))
        papool = ctx.enter_context(tc.tile_pool(name="pacc", bufs=1))
        fpool = ctx.enter_context(tc.tile_pool(name="fin", bufs=2))
        fpool1 = ctx.enter_context(tc.tile_pool(name="fin1", bufs=1))
        vspool = ctx.enter_context(tc.tile_pool(name="vts", bufs=2))

        # ---------------- constants / biases ----------------
        ident = cpool.tile([128, 128], F32)
        masks.make_identity(nc, ident[:])
        bq1 = cpool.tile([128, 1], F32, tag="bq1")
        bq2 = cpool.tile([128, 1], F32, tag="bq2")
        bk1 = cpool.tile([128, 1], F32, tag="bk1")
        bk2 = cpool.tile([128, 1], F32, tag="bk2")
        bvt = cpool.tile([128, 1], F32, tag="bvt")
        lam_t = cpool.tile([128, 1], F32, tag="lam")
        nc.gpsimd.dma_start(out=bq1[:], in_=bq[0:128, :])
        nc.gpsimd.dma_start(out=bq2[:], in_=bq[128:256, :])
        nc.gpsimd.dma_start(out=bk1[:], in_=bk[0:128, :])
        nc.gpsimd.dma_start(out=bk2[:], in_=bk[128:256, :])
        nc.gpsimd.dma_start(out=bvt[:], in_=bv[0:128, :])
        nc.gpsimd.dma_start(out=lam_t[:], in_=lamv[:, :])

        ones_f = cpool.tile([128, 1], F32, tag="ones_f")
        nc.vector.memset(ones_f[:], 1.0)
        ones_rf = cpool.tile([1, 128], F32, tag="ones_rf")
        nc.vector.memset(ones_rf[:], 1.0)
        ones_col = cpool.tile([128, 1], MM_DT, tag="ones_col")
        nc.vector.tensor_copy(ones_col[:], ones_f[:])
        ones_row = cpool.tile([1, 128], MM_DT, tag="ones_row")
        nc.vector.tensor_copy(ones_row[:], ones_rf[:])

        # epilogue scalars: neg_lam = -lam ; bvl = (1-lam)*bv
        neg_lam = cpool.tile([128, 1], F32, tag="neg_lam")
        oml = cpool.tile([128, 1], F32, tag="oml")
        bvl = cpool.tile([128, 1], F32, tag="bvl")
        nc.vector.tensor_scalar_mul(neg_lam[:], lam_t[:], -1.0)
        nc.vector.tensor_scalar(oml[:], lam_t[:], -1.0, 1.0, ALU.mult, ALU.add)
        nc.vector.tensor_mul(bvl[:], bvt[:], oml[:])

        # ---------------- weights ----------------
        # wk1 as 8 separate slice tiles so the very first matmul only waits
        # for one 64KB weight slice + one 256KB X slice.
        wk1s = [
            wpool.tile([128, 128], MM_DT, tag=f"wk1_{e}", name=f"wk1_{e}")
            for e in range(NE)
        ]
        wq1 = wpool.tile([128, NE, 128], MM_DT, tag="wq1")
        wq2 = wpool.tile([128, NE, 128], MM_DT, tag="wq2")
        wk2 = wpool.tile([128, NE, 128], MM_DT, tag="wk2")
        wvt = wpool.tile([128, NE, 128], MM_DT, tag="wvt")

        def wsrc(w, dsl):
            return _in(w[:, dsl]).rearrange("(t p) d -> p t d", p=128)

        qt1 = qkv.tile([128, NQ], MM_DT, tag="qt1")
        qt2 = qkv.tile([128, NQ], MM_DT, tag="qt2")
        kt1 = qkv.tile([128, S], MM_DT, tag="kt1")
        kt2 = qkv.tile([128, S], MM_DT, tag="kt2")
        vv = qkv.tile([128, S], MM_DT, tag="vv")  # V[key, d]

        # ---------------- attention state ----------------
        st = {}

        def attn_begin(sup, comp, spool):
            st[(sup, comp)] = dict(
                o=opool.tile([128, SUP], F32, tag="o", name=f"o{comp}_{sup}"),
                # independent per-engine accumulators: a single pacc would
                # serialize DVE->Pool->DVE with ~0.3us cross-engine latency
                # per link, pacing the whole pass
                pacc_d=papool.tile(
                    [128, SUP], MM_DT, tag=f"pd{comp}", name=f"pd{comp}_{sup}"
                ),
                pacc_p=papool.tile(
                    [128, SUP], MM_DT, tag=f"pp{comp}", name=f"pp{comp}_{sup}"
                ),
                spool=spool,
                s_prev=None,
                e_prev=None,
            )

        def attn_scores(sup, comp, k):
            ktc, qtc = (kt1, qt1) if comp == 1 else (kt2, qt2)
            s_t = st[(sup, comp)]
            # early-exp: consume the previous score tile before reallocating
            if s_t["s_prev"] is not None:
                e_t = epool.tile([128, SUP], MM_DT, tag="e", name=f"e{comp}_{sup}_{k-1}")
                nc.scalar.activation(
                    e_t[:], s_t["s_prev"][:], AF.Exp, scale=float(INV_SQRT_D)
                )
                s_t["e_prev"] = e_t
            s = s_t["spool"].tile(
                [128, SUP], F32, tag="s", name=f"s{comp}_{sup}_{k}"
            )
            ksl = slice(k * 128, (k + 1) * 128)
            for h in range(2):
                hsl = slice(h * 512, (h + 1) * 512)
                nc.tensor.matmul(
                    s[:, hsl],
                    ktc[:, ksl],
                    qtc[:, sup * SUP + h * 512 : sup * SUP + (h + 1) * 512],
                    start=True,
                    stop=True,
                )
            s_t["s_prev"] = s

        def attn_consume(sup, comp, k):
            """accumulate + PV for iteration k (e_prev must hold exp(k))."""
            s_t = st[(sup, comp)]
            e_t = s_t["e_prev"]
            if k == 0:
                nc.vector.tensor_copy(s_t["pacc_d"][:], _f32(e_t[:]))
            elif k % 8 != 0:
                nc.vector.tensor_add(
                    s_t["pacc_d"][:], _f32(s_t["pacc_d"][:]), _f32(e_t[:])
                )
            elif k == 8:
                nc.gpsimd.tensor_copy(s_t["pacc_p"][:], _f32(e_t[:]))
            else:
                nc.gpsimd.tensor_add(
                    s_t["pacc_p"][:], _f32(s_t["pacc_p"][:]), _f32(e_t[:])
                )
            ksl = slice(k * 128, (k + 1) * 128)
            o_ps = s_t["o"]
            for h in range(2):
                hsl = slice(h * 512, (h + 1) * 512)
                nc.tensor.matmul(
                    o_ps[:, hsl],
                    vv[:, ksl],
                    e_t[:, hsl],
                    start=(k == 0),
                    stop=(k == NSK - 1),
                )

        def attn_drain(sup, comp):
            s_t = st[(sup, comp)]
            e_t = epool.tile([128, SUP], MM_DT, tag="e", name=f"e{comp}_{sup}_31")
            nc.scalar.activation(
                e_t[:], s_t["s_prev"][:], AF.Exp, scale=float(INV_SQRT_D)
            )
            s_t["e_prev"] = e_t
            s_t["s_prev"] = None
            attn_consume(sup, comp, NSK - 1)

        # finalize steps (trickled into the next pass); denominators via
        # fp32 ones-matmuls into spare score-pool PSUM slots (a gpsimd
        # partition_all_reduce needs MODIFY_POOL_CONFIG barriers that stall
        # every engine for ~11us)
        fin = {}

        def fin_evac(sup, comp):
            """Copy the PV accumulator out of PSUM so the o slot frees."""
            s_t = st[(sup, comp)]
            o_s = fpool.tile([128, SUP], F32, tag="oev", name=f"oev{comp}_{sup}")
            nc.vector.tensor_copy(o_s[:], s_t["o"][:])
            s_t["o_s"] = o_s

        def fin_rs(sup, comp, spool):
            s_t = st[(sup, comp)]
            rs = spool.tile([1, SUP], F32, tag="s", name=f"rs{comp}_{sup}")
            for h in range(2):
                hsl = slice(h * 512, (h + 1) * 512)
                nc.tensor.matmul(
                    rs[0:1, hsl], ones_col[:], s_t["pacc_d"][:, hsl],
                    start=True, stop=False,
                )
                nc.tensor.matmul(
                    rs[0:1, hsl], ones_col[:], s_t["pacc_p"][:, hsl],
                    start=False, stop=True,
                )
            fin[(sup, comp, "rs")] = rs

        def fin_recip(sup, comp):
            rs = fin.pop((sup, comp, "rs"))
            r = fpool1.tile([1, SUP], F32, tag="r", name=f"r{comp}_{sup}")
            nc.vector.reciprocal_approx_fast(out=r[0:1, :], in_=rs[0:1, :])
            rr = fpool1.tile([1, SUP], MM_DT, tag="rr", name=f"rr{comp}_{sup}")
            nc.vector.tensor_copy(rr[0:1, :], r[0:1, :])
            fin[(sup, comp, "r")] = rr

        def fin_ib(sup, comp, spool):
            r = fin.pop((sup, comp, "r"))
            ib = spool.tile([128, SUP], F32, tag="s", name=f"ib{comp}_{sup}")
            for h in range(2):
                hsl = slice(h * 512, (h + 1) * 512)
                nc.tensor.matmul(
                    ib[:, hsl], ones_row[:], r[0:1, hsl], start=True, stop=True
                )
            fin[(sup, comp, "ib")] = ib

        def fin_t(sup, comp):
            ib = fin.pop((sup, comp, "ib"))
            s_t = st.pop((sup, comp))
            t = fpool1.tile([128, SUP], F32, tag=f"t{comp}", name=f"t{comp}_{sup}")
            nc.vector.tensor_mul(t[:], s_t["o_s"][:], ib[:])
            if comp == 1:
                # fold the V-bias term (1-lam)*bv here, off the critical tail
                nc.vector.tensor_scalar_add(t[:], t[:], bvl[:, 0:1])
            fin[(sup, comp, "t")] = t

        def fin_out(sup):
            t1 = fin.pop((sup, 1, "t"))
            t2 = fin.pop((sup, 2, "t"))
            oo = fpool1.tile([128, SUP], F32, tag="oo", name=f"oo_{sup}")
            # oo = (t2 * -lam) + t1   (t1 already carries (1-lam)*bv)
            nc.vector.scalar_tensor_tensor(
                out=oo[:],
                in0=t2[:],
                scalar=neg_lam[:, 0:1],
                in1=t1[:],
                op0=ALU.mult,
                op1=ALU.add,
            )
            nc.sync.dma_start(
                out=out[:, sup * SUP : (sup + 1) * SUP], in_=oo[:]
            )

        # ---------------- merged proj + attention ----------------
        opool = ctx.enter_context(tc.tile_pool(name="op", bufs=1, space="PSUM"))

        with ExitStack() as pctx:
            spoolM = pctx.enter_context(tc.tile_pool(name="sM", bufs=1, space="PSUM"))
            ppool = pctx.enter_context(tc.tile_pool(name="pj", bufs=2, space="PSUM"))
            tpool = pctx.enter_context(tc.tile_pool(name="tr", bufs=2, space="PSUM"))
            x0pool = pctx.enter_context(tc.tile_pool(name="x0", bufs=1))

            # startup: wk1 slices + first-chunk slices, finest first
            x0s = [
                x0pool.tile([128, SQC], MM_DT, tag=f"x0_{e}", name=f"x0_{e}")
                for e in range(NE)
            ]
            for e in range(NE):
                r = slice(e * 128, (e + 1) * 128)
                nc.sync.dma_start(out=wk1s[e][:], in_=_in(wk[r, 0:128]))
                nc.sync.dma_start(out=x0s[e][:], in_=_in(xt[r, 0:SQC]))
            nc.gpsimd.dma_start(out=wk2[:], in_=wsrc(wk, slice(128, 256)))
            nc.gpsimd.dma_start(out=wq1[:], in_=wsrc(wq, slice(0, 128)))
            nc.gpsimd.dma_start(out=wvt[:], in_=wsrc(wv, slice(0, 128)))
            nc.gpsimd.dma_start(out=wq2[:], in_=wsrc(wq, slice(128, 256)))

            attn_begin(0, 1, spoolM)

            def xsl(xt_t, e):
                return xt_t[e][:, :] if isinstance(xt_t, list) else xt_t[:, e, :]

            def wslice(w_t, e):
                return w_t[e][:, :] if isinstance(w_t, list) else w_t[:, e, :]

            def proj_group(xt_t, dst, w_t, b_t, tag, cc, csl):
                ps = ppool.tile([128, SQC], F32, tag="pj", name=f"ps_{tag}_{cc}")
                for t in range(NE):
                    nc.tensor.matmul(
                        ps[:],
                        wslice(w_t, t),
                        xsl(xt_t, t),
                        start=(t == 0),
                        stop=(t == NE - 1),
                    )
                nc.vector.tensor_scalar_add(dst[:, csl], ps[:], b_t[:, 0:1])

            def proj_v(xt_t, cc, csl):
                ps = ppool.tile([128, SQC], F32, tag="pj", name=f"ps_vt_{cc}")
                for t in range(NE):
                    nc.tensor.matmul(
                        ps[:],
                        wslice(wvt, t),
                        xsl(xt_t, t),
                        start=(t == 0),
                        stop=(t == NE - 1),
                    )
                vt_s = vspool.tile([128, SQC], F32, tag="vts", name=f"vts_{cc}")
                nc.vector.tensor_copy(vt_s[:], ps[:])
                for j in range(SQC // 128):
                    tr = tpool.tile([128, 128], F32, tag="vtr", name=f"vtr_{cc}_{j}")
                    nc.tensor.transpose(
                        tr[:], vt_s[:, j * 128 : (j + 1) * 128], ident[:]
                    )
                    col = (cc * (SQC // 128) + j) * 128
                    nc.vector.tensor_copy(vv[:, col : col + 128], tr[:])

            xts = {0: x0s}

            def issue_chunk_dma(cc):
                # prefetch one chunk ahead (xpool bufs=2 bounds the depth)
                if cc >= NCC or cc in xts:
                    return
                t = xpool.tile([128, NE, SQC], MM_DT, tag="xchunk", name=f"xc_{cc}")
                csl_c = slice(cc * SQC, (cc + 1) * SQC)
                nc.sync.dma_start(
                    out=t[:],
                    in_=_in(xt[:, csl_c]).rearrange("(t p) s -> p t s", p=128),
                )
                xts[cc] = t

            issue_chunk_dma(1)
            for cc in range(NCC):
                csl = slice(cc * SQC, (cc + 1) * SQC)
                xt_t = xts.pop(cc)
                issue_chunk_dma(cc + 1)

                units = [
                    lambda t=t: proj_group(xt_t, *t, cc, csl)
                    for t in (
                        [(kt1, wk1s, bk1, "k1"), (kt2, wk2, bk2, "k2")]
                        + (
                            [(qt1, wq1, bq1, "q1"), (qt2, wq2, bq2, "q2")]
                            if cc < NQ // SQC
                            else []
                        )
                    )
                ] + [lambda: proj_v(xt_t, cc, csl)]

                def attn_iter(k):
                    attn_scores(0, 1, k)
                    if k > 0:
                        attn_consume(0, 1, k - 1)

                iters = (
                    [lambda k=k: attn_iter(k) for k in range(4 * (cc - 1), 4 * cc)]
                    if cc >= 1
                    else []
                )
                # interleave proj units with attention iterations; for cc==1
                # the q1/q2 evacuations must precede the first scores that
                # read them, so attention starts only after three units
                lead = 3 if cc == 1 else 1
                seq = units[:lead]
                rest = units[lead:]
                for j in range(max(len(rest), len(iters))):
                    if j < len(iters):
                        seq.append(iters[j])
                    if j < len(rest):
                        seq.append(rest[j])
                for f in seq:
                    f()

            for k in range(4 * (NCC - 1), NSK):
                attn_scores(0, 1, k)
                attn_consume(0, 1, k - 1)
            attn_drain(0, 1)

        # ---------------- remaining passes ----------------
        # triple-buffered score PSUM shared by all passes: scores(k) only
        # waits on exp(k-3), keeping the PE ahead of the ACT queue
        spoolS = ctx.enter_context(tc.tile_pool(name="sS", bufs=3, space="PSUM"))

        def run_pass(sup, comp, prev, last=False):
            # free the previous pass's o slot before reallocating it
            if prev is not None:
                fin_evac(*prev)
            attn_begin(sup, comp, spoolS)
            trickle = (
                {
                    3: lambda: fin_rs(*prev, spoolS),
                    5: lambda: fin_recip(*prev),
                    8: lambda: fin_ib(*prev, spoolS),
                    10: lambda: fin_t(*prev),
                    12: (lambda: fin_out(prev[0])) if prev[1] == 2 else None,
                }
                if prev is not None
                else {}
            )
            for k in range(NSK):
                attn_scores(sup, comp, k)
                if k > 0:
                    attn_consume(sup, comp, k - 1)
                f = trickle.get(k)
                if f is not None:
                    f()
            attn_drain(sup, comp)

        run_pass(0, 2, prev=(0, 1))  # phase A; finalize comp1/sup0 behind it
        run_pass(1, 1, prev=(0, 2))  # phase B; finalize comp2/sup0 + output
        run_pass(1, 2, prev=(1, 1))  # phase C; finalize comp1/sup1
        # tail: finalize comp2/sup1 in independent halves so the serial
        # rs->recip->ib->mul->sub chain pipelines across PE/DVE
        s_t2 = st.pop((1, 2))
        o_s2 = fpool.tile([128, SUP], F32, tag="oev", name="oev2_1t")
        t1f = fin.pop((1, 1, "t"))
        rs_t = spoolS.tile([1, SUP], F32, tag="s", name="rs_tail")
        ib_t = spoolS.tile([128, SUP], F32, tag="s", name="ib_tail")
        for h in range(2):
            hsl = slice(h * 512, (h + 1) * 512)
            nc.vector.tensor_copy(o_s2[:, hsl], s_t2["o"][:, hsl])
            nc.tensor.matmul(
                rs_t[0:1, hsl], ones_col[:], s_t2["pacc_d"][:, hsl],
                start=True, stop=False,
            )
            nc.tensor.matmul(
                rs_t[0:1, hsl], ones_col[:], s_t2["pacc_p"][:, hsl],
                start=False, stop=True,
            )
            r_h = fpool1.tile([1, 512], F32, tag="r", name=f"rt_{h}")
            nc.vector.reciprocal_approx_fast(out=r_h[0:1, :], in_=rs_t[0:1, hsl])
            rr_h = fpool1.tile([1, 512], MM_DT, tag="rr", name=f"rrt_{h}")
            nc.vector.tensor_copy(rr_h[0:1, :], r_h[0:1, :])
            nc.tensor.matmul(
                ib_t[:, hsl], ones_row[:], rr_h[0:1, :], start=True, stop=True
            )
            t2_h = fpool1.tile([128, 512], F32, tag="t2", name=f"t2t_{h}")
            nc.vector.tensor_mul(t2_h[:], o_s2[:, hsl], ib_t[:, hsl])
            oo_h = fpool1.tile([128, 512], F32, tag="oo", name=f"oot_{h}")
            nc.vector.scalar_tensor_tensor(
                out=oo_h[:],
                in0=t2_h[:],
                scalar=neg_lam[:, 0:1],
                in1=t1f[:, hsl],
                op0=ALU.mult,
                op1=ALU.add,
            )
            nc.sync.dma_start(
                out=out[:, SUP + h * 512 : SUP + (h + 1) * 512], in_=oo_h[:]
            )

    nc.compile()
    return nc


_NC = None


def _get_nc():
    global _NC
    if _NC is None:
        _NC = _build()
    return _NC


def kernel(X, lam, Wq, bq, Wk, bk, Wv, bv):
    X = np.asarray(X, dtype=np.float32)
    lam_f = float(np.asarray(lam))
    Wq = np.ascontiguousarray(np.asarray(Wq, np.float32))
    Wk = np.ascontiguousarray(np.asarray(Wk, np.float32))
    Wv = np.ascontiguousarray(np.asarray(Wv, np.float32))
    bq_c = np.asarray(bq, np.float32).reshape(2 * D, 1).copy()
    bk_c = np.asarray(bk, np.float32).reshape(2 * D, 1).copy()
    bv_c = np.asarray(bv, np.float32).reshape(D, 1).copy()
    lam_v = np.full((128, 1), lam_f, np.float32)

    nc = _get_nc()

    in_maps = []
    for core in range(8):
        b, h = divmod(core, 2)
        xb = X[b]
        if h == 0:
            xr = xb
        else:
            xr = np.concatenate([xb[NQ:], xb[:NQ]], axis=0)
        xt_a = np.ascontiguousarray(xr.T)
        in_maps.append(
            {
                "xt": xt_a,
                "wq": Wq,
                "wk": Wk,
                "wv": Wv,
                "bq": bq_c,
                "bk": bk_c,
                "bv": bv_c,
                "lamv": lam_v,
            }
        )

    global LAST_RESULT
    kwargs = {}
    if TRACE:
        import tempfile

        tdir = tempfile.mkdtemp(dir=TRACE_DIR) if TRACE_DIR else None
        kwargs = dict(trace=True, tmpdir=tdir)
    res = run_bass_kernel_spmd(nc, in_maps, list(range(8)), **kwargs)
    LAST_RESULT = res

    o = np.empty((B, S, D), np.float32)
    for core in range(8):
        b, h = divmod(core, 2)
        o[b, h * NQ : (h + 1) * NQ, :] = res.results[core]["o"].T
    return o
